# revision 21
# baseline (speedup 1.0000x reference)
"""Dynamic 3x3 per-pixel filter (DynamicFilterLayer2D) on 8 Trainium2 cores.

Reference: out[b,c,h,w] = sum_{i,j in 3x3} xpad[b,c,h+i,w+j] * f[b,c,(3i+j),h,w]

Sharding: H split into 8 bands of 32 rows per core (data parallel, 1-row
halo). Per-core layout: partitions = 128 (b,c) images (2 groups), free dim
= flat pixels.

Compute (per super-tile of rd rows = rd*256 pixels):
  1. DVE: 9 bf16 tensor_tensor multiplies prod_t = x_shifted * f_t, one per
     tap, planar. All APs are step-1/4B-aligned so the DVE runs in 2x_1p
     packed mode. The three center-column taps (j=1, odd element shift)
     instead multiply an UNSHIFTED x window by a filter plane the host
     pre-shifted one element right (g_t[q] = f_t[q-1]); the +1 realignment
     happens at the PE read below.
  2. DVE: two in-place pair-adds fold taps (0,0)+(0,2) and (1,0)+(1,2),
     balancing DVE vs PE stage time.
  3. PE: 7 accumulating identity matmuls per 512-px PSUM bank sum the
     remaining planes into fp32 PSUM (tensor engine is otherwise idle);
     pre-shifted planes are read at +1 element offset.
  4. ACT: one activation-copy drains PSUM fp32 -> SBUF bf16.
Inputs/outputs travel as bf16 (halves HBM traffic; rel err ~3e-3), the tap
sum stays fp32 in PSUM. Filter border columns (taps that multiply x-padding
or row-wrapped elements) are zeroed host-side. Filters are pre-packed
host-side into one contiguous slab per DMA: normal taps [0,2,3,5,6,8] on
the sync HWDGE ring, shifted taps [1,4,7] on the scalar HWDGE ring.
"""

import numpy as np
import ml_dtypes

B, C, H, W = 8, 32, 256, 256
K = 3
N_CORES = 8
BAND = H // N_CORES            # 32 rows per core
N_IMG = B * C                  # 256 images
P = 128
GROUPS = N_IMG // P            # 2
FD = 8 * W                     # pixels per full super-tile (2048)
XW = FD + 2 * W + 2            # x elements per super-tile incl halo+guards
X_FLAT = (BAND + 2) * W + 2    # per-image padded x row storage
BANK = 512                     # PSUM bank capacity in fp32 elements
NTAP = K * K
SUPER_SCHED = [(0, 4), (4, 8), (12, 8), (20, 8), (28, 4)]
NORM_TAPS = [0, 3, 2, 5, 6, 8]   # even j: x read at aligned offset i*W+j
SHIFT_TAPS = [1, 4, 7]           # j=1: host-shifted filter plane, PE +1

_CACHE = {}


def _build_module():
    import concourse.bacc as bacc
    import concourse.mybir as mybir
    from concourse.tile import TileContext

    bf16 = mybir.dt.bfloat16
    fp32 = mybir.dt.float32
    mult = mybir.AluOpType.mult
    add = mybir.AluOpType.add

    nc = bacc.Bacc("TRN2", target_bir_lowering=False, debug=False)
    x_d = nc.dram_tensor("x_s", [N_IMG, X_FLAT], bf16,
                         kind="ExternalInput").ap()
    fwidth = sum(6 * rd * W + 3 * (rd * W + 2) for (_, rd) in SUPER_SCHED)
    f_d = nc.dram_tensor("f_s", [N_IMG, fwidth], bf16,
                         kind="ExternalInput").ap()
    i_d = nc.dram_tensor("ident", [P, P], bf16, kind="ExternalInput").ap()
    o_d = nc.dram_tensor("o_s", [N_IMG, BAND, W], bf16,
                         kind="ExternalOutput").ap()

    with TileContext(nc) as tc:
        with (
            tc.tile_pool(name="ip", bufs=1) as ipool,
            tc.tile_pool(name="xp", bufs=2) as xpool,
            tc.tile_pool(name="fsp", bufs=3) as fspool,
            tc.tile_pool(name="pr", bufs=2) as prpool,
            tc.tile_pool(name="op", bufs=2) as opool,
            tc.tile_pool(name="ps", bufs=2, space="PSUM") as pspool,
        ):
            ident = ipool.tile([P, P], bf16)
            nc.sync.dma_start(out=ident[:], in_=i_d[:, :])
            n_sup = len(SUPER_SCHED) * GROUPS
            i_sup = 0
            for g in range(GROUPS):
                p0 = g * P
                foff = 0
                for (r0, rd) in SUPER_SCHED:
                    i_sup += 1
                    last = i_sup == n_sup
                    fd = rd * W
                    fds = fd + 2            # shifted plane width
                    xw = fd + 2 * W + 2
                    xt = xpool.tile([P, XW], bf16, tag="x")
                    nc.gpsimd.dma_start(
                        out=xt[:, 0:xw],
                        in_=x_d[p0:p0 + P, r0 * W: r0 * W + xw])
                    # one big f DMA per super, alternating HWDGE rings:
                    # bigger DMAs run the rings faster, and the two rings
                    # stream adjacent supers concurrently
                    fts = fspool.tile([P, 6 * FD + 3 * (FD + 2)], bf16,
                                      tag="fs")
                    f_eng = nc.sync if i_sup % 2 else nc.scalar
                    f_eng.dma_start(
                        out=fts[:, 0:6 * fd + 3 * fds],
                        in_=f_d[p0:p0 + P, foff: foff + 6 * fd + 3 * fds])
                    ftc = fts
                    foff += 6 * fd + 3 * fds
                    # prod planes: 6 normal (width fd) then 3 shifted
                    # (width fd+2)
                    pr = prpool.tile([P, 6 * FD + 3 * (FD + 2)], bf16,
                                     tag="pr")
                    for n, t in enumerate(NORM_TAPS):
                        i, j = divmod(t, K)
                        nc.vector.tensor_tensor(
                            pr[:, n * fd:(n + 1) * fd],
                            xt[:, i * W + j: i * W + j + fd],
                            fts[:, n * fd:(n + 1) * fd], mult)
                    sbase = 6 * fd
                    for n, t in enumerate(SHIFT_TAPS):
                        i = t // K
                        nc.vector.tensor_tensor(
                            pr[:, sbase + n * fds: sbase + (n + 1) * fds],
                            xt[:, i * W: i * W + fds],
                            ftc[:, sbase + n * fds: sbase + (n + 1) * fds],
                            mult)
                    # fold taps (0,0)+(0,2) and (1,0)+(1,2) in one add:
                    # plane order [t0, t3, t2, t5] makes both pairs adjacent
                    nc.vector.tensor_tensor(
                        pr[:, 0:2 * fd], pr[:, 0:2 * fd],
                        pr[:, 2 * fd:4 * fd], add)
                    ps = pspool.tile([P, FD], fp32, tag="ps")
                    for b in range(fd // BANK):
                        srcs = [pr[:, n * fd + b * BANK:
                                   n * fd + b * BANK + BANK]
                                for n in (0, 1, 4, 5)]
                        srcs += [pr[:, sbase + n * fds + b * BANK + 1:
                                    sbase + n * fds + b * BANK + 1 + BANK]
                                 for n in range(3)]
                        for n, src in enumerate(srcs):
                            nc.tensor.matmul(
                                ps[:, b * BANK:(b + 1) * BANK],
                                ident[:], src,
                                start=(n == 0), stop=(n == len(srcs) - 1))
                    ot = opool.tile([P, FD], bf16, tag="o")
                    nc.scalar.copy(ot[:, 0:fd], ps[:, 0:fd])
                    # stores ride the idle SWDGE ring; the final one goes on
                    # the (by then empty) scalar HWDGE ring for a fast finish
                    out_eng = nc.scalar if last else nc.gpsimd
                    out_eng.dma_start(
                        out=o_d[p0:p0 + P, r0:r0 + rd, :], in_=ot[:, 0:fd])
    nc.compile()
    return nc


def _get_module():
    if "nc" not in _CACHE:
        _CACHE["nc"] = _build_module()
    return _CACHE["nc"]


def _shard_inputs(x, dynamic_filters):
    """Per-core input maps. x: [B,C,H,W] f32, filters: [B,C*9,H,W] f32."""
    bf = ml_dtypes.bfloat16
    xb = x.astype(bf)
    xp = np.pad(xb, ((0, 0), (0, 0), (1, 1), (0, 0)))   # pad rows only
    f6 = dynamic_filters.reshape(B, C, K, K, H, W).astype(bf)
    f6[:, :, :, 0, :, 0] = 0      # j=0 taps multiply x col -1
    f6[:, :, :, 2, :, W - 1] = 0  # j=2 taps multiply x col W
    ftap = f6.reshape(N_IMG, NTAP, H, W)
    ident = np.eye(P, dtype=bf)
    fwidth = sum(6 * rd * W + 3 * (rd * W + 2) for (_, rd) in SUPER_SCHED)

    in_maps = []
    for n in range(N_CORES):
        r = n * BAND
        xs = xp[:, :, r:r + BAND + 2, :].reshape(N_IMG, (BAND + 2) * W)
        xs_flat = np.zeros((N_IMG, X_FLAT), bf)
        xs_flat[:, 1:-1] = xs
        fband = np.ascontiguousarray(ftap[:, :, r:r + BAND]).reshape(
            N_IMG, NTAP, BAND * W)
        fpad = np.pad(fband, ((0, 0), (0, 0), (1, 1)))
        fs = np.empty((N_IMG, fwidth), bf)
        foff = 0
        for (r0, rd) in SUPER_SCHED:
            fd = rd * W
            fds = fd + 2
            blk = fband[:, NORM_TAPS, r0 * W: r0 * W + fd]
            fs[:, foff: foff + 6 * fd] = blk.reshape(N_IMG, -1)
            # shifted planes: g_t[q] = f_t[q-1] over q in [0, fd+2)
            sblk = fpad[:, SHIFT_TAPS, r0 * W: r0 * W + fds]
            fs[:, foff + 6 * fd: foff + 6 * fd + 3 * fds] = \
                sblk.reshape(N_IMG, -1)
            foff += 6 * fd + 3 * fds
        in_maps.append({"x_s": xs_flat, "f_s": fs, "ident": ident})
    return in_maps


def kernel(x, dynamic_filters, _trace=False):
    from concourse import bass_utils

    x = np.asarray(x, dtype=np.float32)
    dynamic_filters = np.asarray(dynamic_filters, dtype=np.float32)
    nc = _get_module()
    in_maps = _shard_inputs(x, dynamic_filters)
    res = bass_utils.run_bass_kernel_spmd(
        nc, in_maps, list(range(N_CORES)), trace=_trace)
    out = np.concatenate(
        [np.asarray(res.results[n]["o_s"]).reshape(B, C, BAND, W)
         for n in range(N_CORES)],
        axis=2).astype(np.float32)
    _CACHE["last_exec_time_ns"] = res.exec_time_ns
    if res.instructions_and_trace is not None:
        _CACHE["trace_path"] = res.instructions_and_trace[1]
    return out


# revision 25
# speedup vs baseline: 1.1792x; 1.1792x over previous
"""Dynamic 3x3 per-pixel filter (DynamicFilterLayer2D) on 8 Trainium2 cores.

Reference: out[b,c,h,w] = sum_{i,j in 3x3} xpad[b,c,h+i,w+j] * f[b,c,(3i+j),h,w]

Sharding: H split into 8 bands of 32 rows per core (data parallel, 1-row
halo). Per-core layout: partitions = 128 (b,c) images (2 groups), free dim
= flat pixels.

Compute (per super-tile of rd rows = rd*256 pixels):
  1. DVE: 9 bf16 tensor_tensor multiplies prod_t = x_shifted * f_t, one per
     tap, planar. All APs are step-1/4B-aligned so the DVE runs in 2x_1p
     packed mode. The three center-column taps (j=1, odd element shift)
     instead multiply an UNSHIFTED x window by a filter plane the host
     pre-shifted one element right (g_t[q] = f_t[q-1]); the +1 realignment
     happens at the PE read below.
  2. DVE: two in-place pair-adds fold taps (0,0)+(0,2) and (1,0)+(1,2),
     balancing DVE vs PE stage time.
  3. PE: 7 accumulating identity matmuls per 512-px PSUM bank sum the
     remaining planes into fp32 PSUM (tensor engine is otherwise idle);
     pre-shifted planes are read at +1 element offset.
  4. ACT: one activation-copy drains PSUM fp32 -> SBUF bf16.
Inputs/outputs travel as bf16 (halves HBM traffic; rel err ~3e-3), the tap
sum stays fp32 in PSUM. Filter border columns (taps that multiply x-padding
or row-wrapped elements) are zeroed host-side. Filters are pre-packed
host-side into one contiguous slab per DMA: normal taps [0,2,3,5,6,8] on
the sync HWDGE ring, shifted taps [1,4,7] on the scalar HWDGE ring.
"""

import numpy as np
import ml_dtypes

B, C, H, W = 8, 32, 256, 256
K = 3
N_CORES = 8
BAND = H // N_CORES            # 32 rows per core
N_IMG = B * C                  # 256 images
P = 128
GROUPS = N_IMG // P            # 2
FD = 8 * W                     # pixels per full super-tile (2048)
XW = FD + 2 * W + 2            # x elements per super-tile incl halo+guards
X_FLAT = (BAND + 2) * W + 2    # per-image padded x row storage
BANK = 512                     # PSUM bank capacity in fp32 elements
NTAP = K * K
SUPER_SCHED = [(0, 4), (4, 8), (12, 8), (20, 8), (28, 4)]
NORM_TAPS = [0, 3, 2, 5, 6, 8]   # even j: x read at aligned offset i*W+j
SHIFT_TAPS = [1, 4, 7]           # j=1: host-shifted filter plane, PE +1

_CACHE = {}


def _build_module():
    import concourse.bacc as bacc
    import concourse.mybir as mybir
    from concourse.tile import TileContext

    bf16 = mybir.dt.bfloat16
    fp32 = mybir.dt.float32
    mult = mybir.AluOpType.mult
    add = mybir.AluOpType.add

    nc = bacc.Bacc("TRN2", target_bir_lowering=False, debug=False)
    x_d = nc.dram_tensor("x_s", [N_IMG, X_FLAT], bf16,
                         kind="ExternalInput").ap()
    fwidth = sum(6 * rd * W + 3 * (rd * W + 2) for (_, rd) in SUPER_SCHED)
    f_d = nc.dram_tensor("f_s", [N_IMG, fwidth], bf16,
                         kind="ExternalInput").ap()
    i_d = nc.dram_tensor("ident", [P, P], bf16, kind="ExternalInput").ap()
    o_d = nc.dram_tensor("o_s", [N_IMG, BAND, W], bf16,
                         kind="ExternalOutput").ap()

    with TileContext(nc) as tc:
        with (
            tc.tile_pool(name="ip", bufs=1) as ipool,
            tc.tile_pool(name="xp", bufs=2) as xpool,
            tc.tile_pool(name="fsp", bufs=3) as fspool,
            tc.tile_pool(name="fhp", bufs=3) as fhpool,
            tc.tile_pool(name="pr", bufs=2) as prpool,
            tc.tile_pool(name="op", bufs=2) as opool,
            tc.tile_pool(name="ps", bufs=2, space="PSUM") as pspool,
        ):
            ident = ipool.tile([P, P], bf16)
            nc.sync.dma_start(out=ident[:], in_=i_d[:, :])
            n_sup = len(SUPER_SCHED) * GROUPS
            i_sup = 0
            for g in range(GROUPS):
                p0 = g * P
                foff = 0
                for (r0, rd) in SUPER_SCHED:
                    i_sup += 1
                    last = i_sup == n_sup
                    fd = rd * W
                    fds = fd + 2            # shifted plane width
                    xw = fd + 2 * W + 2
                    xt = xpool.tile([P, XW], bf16, tag="x")
                    nc.gpsimd.dma_start(
                        out=xt[:, 0:xw],
                        in_=x_d[p0:p0 + P, r0 * W: r0 * W + xw])
                    # each HWDGE ring streams ~250 GB/s; split every
                    # super's f ~50/50 across both rings so they run
                    # concurrently: 5 normal planes on sync, 1 normal +
                    # 3 shifted planes on scalar
                    fts = fspool.tile([P, 5 * FD], bf16, tag="fs")
                    nc.sync.dma_start(
                        out=fts[:, 0:5 * fd],
                        in_=f_d[p0:p0 + P, foff: foff + 5 * fd])
                    ftc = fhpool.tile([P, FD + 3 * (FD + 2)], bf16,
                                      tag="fh")
                    nc.scalar.dma_start(
                        out=ftc[:, 0:fd + 3 * fds],
                        in_=f_d[p0:p0 + P,
                                foff + 5 * fd: foff + 6 * fd + 3 * fds])
                    foff += 6 * fd + 3 * fds
                    # prod planes: 6 normal (width fd) then 3 shifted
                    # (width fd+2)
                    pr = prpool.tile([P, 6 * FD + 3 * (FD + 2)], bf16,
                                     tag="pr")
                    for n, t in enumerate(NORM_TAPS):
                        i, j = divmod(t, K)
                        fsrc = (fts[:, n * fd:(n + 1) * fd] if n < 5
                                else ftc[:, 0:fd])
                        nc.vector.tensor_tensor(
                            pr[:, n * fd:(n + 1) * fd],
                            xt[:, i * W + j: i * W + j + fd], fsrc, mult)
                    sbase = 6 * fd
                    for n, t in enumerate(SHIFT_TAPS):
                        i = t // K
                        nc.vector.tensor_tensor(
                            pr[:, sbase + n * fds: sbase + (n + 1) * fds],
                            xt[:, i * W: i * W + fds],
                            ftc[:, fd + n * fds: fd + (n + 1) * fds],
                            mult)
                    # fold taps (0,0)+(0,2) and (1,0)+(1,2) in one add:
                    # plane order [t0, t3, t2, t5] makes both pairs adjacent
                    nc.vector.tensor_tensor(
                        pr[:, 0:2 * fd], pr[:, 0:2 * fd],
                        pr[:, 2 * fd:4 * fd], add)
                    ps = pspool.tile([P, FD], fp32, tag="ps")
                    for b in range(fd // BANK):
                        srcs = [pr[:, n * fd + b * BANK:
                                   n * fd + b * BANK + BANK]
                                for n in (0, 1, 4, 5)]
                        srcs += [pr[:, sbase + n * fds + b * BANK + 1:
                                    sbase + n * fds + b * BANK + 1 + BANK]
                                 for n in range(3)]
                        for n, src in enumerate(srcs):
                            nc.tensor.matmul(
                                ps[:, b * BANK:(b + 1) * BANK],
                                ident[:], src,
                                start=(n == 0), stop=(n == len(srcs) - 1))
                    ot = opool.tile([P, FD], bf16, tag="o")
                    nc.scalar.copy(ot[:, 0:fd], ps[:, 0:fd])
                    # stores ride the idle SWDGE ring; the final one goes on
                    # the (by then empty) scalar HWDGE ring for a fast finish
                    out_eng = nc.scalar if last else nc.gpsimd
                    out_eng.dma_start(
                        out=o_d[p0:p0 + P, r0:r0 + rd, :], in_=ot[:, 0:fd])
    nc.compile()
    return nc


def _get_module():
    if "nc" not in _CACHE:
        _CACHE["nc"] = _build_module()
    return _CACHE["nc"]


def _shard_inputs(x, dynamic_filters):
    """Per-core input maps. x: [B,C,H,W] f32, filters: [B,C*9,H,W] f32."""
    bf = ml_dtypes.bfloat16
    xb = x.astype(bf)
    xp = np.pad(xb, ((0, 0), (0, 0), (1, 1), (0, 0)))   # pad rows only
    f6 = dynamic_filters.reshape(B, C, K, K, H, W).astype(bf)
    f6[:, :, :, 0, :, 0] = 0      # j=0 taps multiply x col -1
    f6[:, :, :, 2, :, W - 1] = 0  # j=2 taps multiply x col W
    ftap = f6.reshape(N_IMG, NTAP, H, W)
    ident = np.eye(P, dtype=bf)
    fwidth = sum(6 * rd * W + 3 * (rd * W + 2) for (_, rd) in SUPER_SCHED)

    in_maps = []
    for n in range(N_CORES):
        r = n * BAND
        xs = xp[:, :, r:r + BAND + 2, :].reshape(N_IMG, (BAND + 2) * W)
        xs_flat = np.zeros((N_IMG, X_FLAT), bf)
        xs_flat[:, 1:-1] = xs
        fband = np.ascontiguousarray(ftap[:, :, r:r + BAND]).reshape(
            N_IMG, NTAP, BAND * W)
        fpad = np.pad(fband, ((0, 0), (0, 0), (1, 1)))
        fs = np.empty((N_IMG, fwidth), bf)
        foff = 0
        for (r0, rd) in SUPER_SCHED:
            fd = rd * W
            fds = fd + 2
            blk = fband[:, NORM_TAPS, r0 * W: r0 * W + fd]
            fs[:, foff: foff + 6 * fd] = blk.reshape(N_IMG, -1)
            # shifted planes: g_t[q] = f_t[q-1] over q in [0, fd+2)
            sblk = fpad[:, SHIFT_TAPS, r0 * W: r0 * W + fds]
            fs[:, foff + 6 * fd: foff + 6 * fd + 3 * fds] = \
                sblk.reshape(N_IMG, -1)
            foff += 6 * fd + 3 * fds
        assert foff == fwidth
        in_maps.append({"x_s": xs_flat, "f_s": fs, "ident": ident})
    return in_maps


def kernel(x, dynamic_filters, _trace=False):
    from concourse import bass_utils

    x = np.asarray(x, dtype=np.float32)
    dynamic_filters = np.asarray(dynamic_filters, dtype=np.float32)
    nc = _get_module()
    in_maps = _shard_inputs(x, dynamic_filters)
    res = bass_utils.run_bass_kernel_spmd(
        nc, in_maps, list(range(N_CORES)), trace=_trace)
    out = np.concatenate(
        [np.asarray(res.results[n]["o_s"]).reshape(B, C, BAND, W)
         for n in range(N_CORES)],
        axis=2).astype(np.float32)
    _CACHE["last_exec_time_ns"] = res.exec_time_ns
    if res.instructions_and_trace is not None:
        _CACHE["trace_path"] = res.instructions_and_trace[1]
    return out


# revision 28
# speedup vs baseline: 1.1884x; 1.0078x over previous
"""Dynamic 3x3 per-pixel filter (DynamicFilterLayer2D) on 8 Trainium2 cores.

Reference: out[b,c,h,w] = sum_{i,j in 3x3} xpad[b,c,h+i,w+j] * f[b,c,(3i+j),h,w]

Sharding: H split into 8 bands of 32 rows per core (data parallel, 1-row
halo). Per-core layout: partitions = 128 (b,c) images (2 groups), free dim
= flat pixels.

Compute (per super-tile of rd rows = rd*256 pixels):
  1. DVE: 9 bf16 tensor_tensor multiplies prod_t = x_shifted * f_t, one per
     tap, planar. All APs are step-1/4B-aligned so the DVE runs in 2x_1p
     packed mode. The three center-column taps (j=1, odd element shift)
     instead multiply an UNSHIFTED x window by a filter plane the host
     pre-shifted one element right (g_t[q] = f_t[q-1]); the +1 realignment
     happens at the PE read below.
  2. DVE: two in-place pair-adds fold taps (0,0)+(0,2) and (1,0)+(1,2),
     balancing DVE vs PE stage time.
  3. PE: 7 accumulating identity matmuls per 512-px PSUM bank sum the
     remaining planes into fp32 PSUM (tensor engine is otherwise idle);
     pre-shifted planes are read at +1 element offset.
  4. ACT: one activation-copy drains PSUM fp32 -> SBUF bf16.
Inputs/outputs travel as bf16 (halves HBM traffic; rel err ~3e-3), the tap
sum stays fp32 in PSUM. Filter border columns (taps that multiply x-padding
or row-wrapped elements) are zeroed host-side. Filters are pre-packed
host-side into one contiguous slab per DMA: normal taps [0,2,3,5,6,8] on
the sync HWDGE ring, shifted taps [1,4,7] on the scalar HWDGE ring.
"""

import numpy as np
import ml_dtypes

B, C, H, W = 8, 32, 256, 256
K = 3
N_CORES = 8
BAND = H // N_CORES            # 32 rows per core
N_IMG = B * C                  # 256 images
P = 128
GROUPS = N_IMG // P            # 2
FD = 8 * W                     # pixels per full super-tile (2048)
XW = FD + 2 * W + 2            # x elements per super-tile incl halo+guards
X_FLAT = (BAND + 2) * W + 2    # per-image padded x row storage
BANK = 512                     # PSUM bank capacity in fp32 elements
NTAP = K * K
SUPER_SCHED = [(0, 4), (4, 8), (12, 8), (20, 8), (28, 4)]
NORM_TAPS = [0, 3, 2, 5, 6, 8]   # even j: x read at aligned offset i*W+j
SHIFT_TAPS = [1, 4, 7]           # j=1: host-shifted filter plane, PE +1

_CACHE = {}


def _build_module():
    import concourse.bacc as bacc
    import concourse.mybir as mybir
    from concourse.tile import TileContext

    bf16 = mybir.dt.bfloat16
    fp32 = mybir.dt.float32
    mult = mybir.AluOpType.mult
    add = mybir.AluOpType.add

    nc = bacc.Bacc("TRN2", target_bir_lowering=False, debug=False)
    x_d = nc.dram_tensor("x_s", [N_IMG, X_FLAT], bf16,
                         kind="ExternalInput").ap()
    fwidth = sum(6 * rd * W + 3 * (rd * W + 2) for (_, rd) in SUPER_SCHED)
    f_d = nc.dram_tensor("f_s", [N_IMG, fwidth], bf16,
                         kind="ExternalInput").ap()
    i_d = nc.dram_tensor("ident", [P, P], bf16, kind="ExternalInput").ap()
    o_d = nc.dram_tensor("o_s", [N_IMG, BAND, W], bf16,
                         kind="ExternalOutput").ap()

    with TileContext(nc) as tc:
        with (
            tc.tile_pool(name="ip", bufs=1) as ipool,
            tc.tile_pool(name="xp", bufs=2) as xpool,
            tc.tile_pool(name="fsp", bufs=3) as fspool,
            tc.tile_pool(name="fhp", bufs=3) as fhpool,
            tc.tile_pool(name="pr", bufs=2) as prpool,
            tc.tile_pool(name="op", bufs=2) as opool,
            tc.tile_pool(name="ps", bufs=2, space="PSUM") as pspool,
        ):
            ident = ipool.tile([P, P], bf16)
            nc.sync.dma_start(out=ident[:], in_=i_d[:, :])
            n_sup = len(SUPER_SCHED) * GROUPS
            i_sup = 0
            for g in range(GROUPS):
                p0 = g * P
                foff = 0
                for (r0, rd) in SUPER_SCHED:
                    i_sup += 1
                    last = i_sup == n_sup
                    fd = rd * W
                    fds = fd + 2            # shifted plane width
                    xw = fd + 2 * W + 2
                    # x rides the sync ring: loads must never queue behind
                    # stores (an out-DMA waits on the pipeline-end drain,
                    # so any load after it on the same engine stalls)
                    xt = xpool.tile([P, XW], bf16, tag="x")
                    nc.sync.dma_start(
                        out=xt[:, 0:xw],
                        in_=x_d[p0:p0 + P, r0 * W: r0 * W + xw])
                    # each HWDGE ring streams ~250 GB/s; split every
                    # super's f across both rings so they run concurrently
                    # (bytes balanced against x on sync)
                    fts = fspool.tile([P, 4 * FD], bf16, tag="fs")
                    nc.sync.dma_start(
                        out=fts[:, 0:4 * fd],
                        in_=f_d[p0:p0 + P, foff: foff + 4 * fd])
                    ftc = fhpool.tile([P, 2 * FD + 3 * (FD + 2)], bf16,
                                      tag="fh")
                    nc.scalar.dma_start(
                        out=ftc[:, 0:2 * fd + 3 * fds],
                        in_=f_d[p0:p0 + P,
                                foff + 4 * fd: foff + 6 * fd + 3 * fds])
                    foff += 6 * fd + 3 * fds
                    # prod planes: 6 normal (width fd) then 3 shifted
                    # (width fd+2)
                    pr = prpool.tile([P, 6 * FD + 3 * (FD + 2)], bf16,
                                     tag="pr")
                    for n, t in enumerate(NORM_TAPS):
                        i, j = divmod(t, K)
                        fsrc = (fts[:, n * fd:(n + 1) * fd] if n < 4
                                else ftc[:, (n - 4) * fd:(n - 3) * fd])
                        nc.vector.tensor_tensor(
                            pr[:, n * fd:(n + 1) * fd],
                            xt[:, i * W + j: i * W + j + fd], fsrc, mult)
                    sbase = 6 * fd
                    for n, t in enumerate(SHIFT_TAPS):
                        i = t // K
                        nc.vector.tensor_tensor(
                            pr[:, sbase + n * fds: sbase + (n + 1) * fds],
                            xt[:, i * W: i * W + fds],
                            ftc[:, 2 * fd + n * fds:
                                2 * fd + (n + 1) * fds], mult)
                    # fold taps (0,0)+(0,2) and (1,0)+(1,2) in one add:
                    # plane order [t0, t3, t2, t5] makes both pairs adjacent
                    nc.vector.tensor_tensor(
                        pr[:, 0:2 * fd], pr[:, 0:2 * fd],
                        pr[:, 2 * fd:4 * fd], add)
                    ps = pspool.tile([P, FD], fp32, tag="ps")
                    for b in range(fd // BANK):
                        srcs = [pr[:, n * fd + b * BANK:
                                   n * fd + b * BANK + BANK]
                                for n in (0, 1, 4, 5)]
                        srcs += [pr[:, sbase + n * fds + b * BANK + 1:
                                    sbase + n * fds + b * BANK + 1 + BANK]
                                 for n in range(3)]
                        for n, src in enumerate(srcs):
                            nc.tensor.matmul(
                                ps[:, b * BANK:(b + 1) * BANK],
                                ident[:], src,
                                start=(n == 0), stop=(n == len(srcs) - 1))
                    ot = opool.tile([P, FD], bf16, tag="o")
                    nc.scalar.copy(ot[:, 0:fd], ps[:, 0:fd])
                    # stores get the SWDGE queue to themselves (they wait on
                    # the drain, so they'd block any load queued after them);
                    # the final one goes on the (by then empty) scalar HWDGE
                    # ring for a fast finish
                    out_eng = nc.scalar if last else nc.gpsimd
                    out_eng.dma_start(
                        out=o_d[p0:p0 + P, r0:r0 + rd, :], in_=ot[:, 0:fd])
    nc.compile()
    return nc


def _get_module():
    if "nc" not in _CACHE:
        _CACHE["nc"] = _build_module()
    return _CACHE["nc"]


def _shard_inputs(x, dynamic_filters):
    """Per-core input maps. x: [B,C,H,W] f32, filters: [B,C*9,H,W] f32."""
    bf = ml_dtypes.bfloat16
    xb = x.astype(bf)
    xp = np.pad(xb, ((0, 0), (0, 0), (1, 1), (0, 0)))   # pad rows only
    f6 = dynamic_filters.reshape(B, C, K, K, H, W).astype(bf)
    f6[:, :, :, 0, :, 0] = 0      # j=0 taps multiply x col -1
    f6[:, :, :, 2, :, W - 1] = 0  # j=2 taps multiply x col W
    ftap = f6.reshape(N_IMG, NTAP, H, W)
    ident = np.eye(P, dtype=bf)
    fwidth = sum(6 * rd * W + 3 * (rd * W + 2) for (_, rd) in SUPER_SCHED)

    in_maps = []
    for n in range(N_CORES):
        r = n * BAND
        xs = xp[:, :, r:r + BAND + 2, :].reshape(N_IMG, (BAND + 2) * W)
        xs_flat = np.zeros((N_IMG, X_FLAT), bf)
        xs_flat[:, 1:-1] = xs
        fband = np.ascontiguousarray(ftap[:, :, r:r + BAND]).reshape(
            N_IMG, NTAP, BAND * W)
        fpad = np.pad(fband, ((0, 0), (0, 0), (1, 1)))
        fs = np.empty((N_IMG, fwidth), bf)
        foff = 0
        for (r0, rd) in SUPER_SCHED:
            fd = rd * W
            fds = fd + 2
            blk = fband[:, NORM_TAPS, r0 * W: r0 * W + fd]
            fs[:, foff: foff + 6 * fd] = blk.reshape(N_IMG, -1)
            # shifted planes: g_t[q] = f_t[q-1] over q in [0, fd+2)
            sblk = fpad[:, SHIFT_TAPS, r0 * W: r0 * W + fds]
            fs[:, foff + 6 * fd: foff + 6 * fd + 3 * fds] = \
                sblk.reshape(N_IMG, -1)
            foff += 6 * fd + 3 * fds
        assert foff == fwidth
        in_maps.append({"x_s": xs_flat, "f_s": fs, "ident": ident})
    return in_maps


def kernel(x, dynamic_filters, _trace=False):
    from concourse import bass_utils

    x = np.asarray(x, dtype=np.float32)
    dynamic_filters = np.asarray(dynamic_filters, dtype=np.float32)
    nc = _get_module()
    in_maps = _shard_inputs(x, dynamic_filters)
    res = bass_utils.run_bass_kernel_spmd(
        nc, in_maps, list(range(N_CORES)), trace=_trace)
    out = np.concatenate(
        [np.asarray(res.results[n]["o_s"]).reshape(B, C, BAND, W)
         for n in range(N_CORES)],
        axis=2).astype(np.float32)
    _CACHE["last_exec_time_ns"] = res.exec_time_ns
    if res.instructions_and_trace is not None:
        _CACHE["trace_path"] = res.instructions_and_trace[1]
    return out
